# revision 2
# baseline (speedup 1.0000x reference)
"""Causal self-attention (B=2,T=2048,C=1024,H=16,hd=64) with QK-RMSNorm + RoPE.

8-core Trainium2 Bass kernel. Sharding: tensor-parallel over heads (2 heads per
core) for QKV + attention, then an AllToAll reshards the attention output
token-wise so each core computes the exact c_proj output for its 512-token
slice (no partial sums, no all-reduce).

Host->device traffic is the measured bottleneck (per-dispatch staging at
~17 GB/s), so every replicated input is sharded on the host and re-replicated
on device over the much faster inter-core links:
  - x ships token-sharded (1MB/core) and is AllGathered on device (two
    feature-half AllGathers so QKV matmuls can start after the first half).
  - w_proj ships row-sharded (0.25MB/core) and is AllGathered on device.
  - cos/sin ship as [32, T] once (not 4x-duplicated rows); the 128-row SBUF
    tiling and sign pattern are built on device.
  - the output is written in fp16 (2e-2 rel tolerance leaves plenty of room).

Layout strategy: everything feature-major ("transposed") on device.
  - per-core waT = w_attn[sel_rows].T so QKV matmuls produce qT/kT/vT
    [feat, tok] with no on-device activation transposes.
  - q,k feature order is permuted to [evens, odds] per head (host-side weight
    row permutation) which turns interleaved RoPE into half-block ops; S = q.k
    is invariant to the shared permutation.
  - S^T tiles [keys,queries] come from lhsT=kT, rhs=qT; softmax denominator is
    computed by a ones-column appended to V (scores are bounded: |s| <= 8
    after RMS-norm, so exp needs no max subtraction).
  - gpsimd runs ONLY collectives (AllGather x2, AllGather wp, AllToAll);
    element-wise work that used to ride gpsimd now rides the vector engine so
    collectives are never queued behind it.
"""

import numpy as np

import concourse.bass as bass
import concourse.mybir as mybir
import concourse.tile as tile
from concourse import bacc
from concourse.bass_utils import run_bass_kernel_spmd

B, T, C = 2, 2048, 1024
H, HD = 16, 64
N_CORES = 8
HPC = H // N_CORES  # heads per core = 2
BT = B * T  # 4096 flattened tokens
FPC = HPC * HD  # feats per core = 128
EPS = 1e-6
TN = BT // 512  # 8 token tiles of 512
QB = T // 512  # 4 query blocks per sequence

f32 = mybir.dt.float32
f32r = mybir.dt.float32r
f16 = mybir.dt.float16
bf16 = mybir.dt.bfloat16
MUL = mybir.AluOpType.mult
ADD = mybir.AluOpType.add
AF = mybir.ActivationFunctionType
BYPASS = mybir.AluOpType.bypass

RG = [list(range(N_CORES))]


def r32(ap):
    return ap.bitcast(f32r)


def build_nc(single_core=False, no_cc=False):
    no_cc = no_cc or single_core
    nc = bacc.Bacc("TRN2", target_bir_lowering=False, debug=False,
                   num_devices=1 if single_core else N_CORES)

    xsh = nc.dram_tensor("xsh", [C, 512], bf16, kind="ExternalInput")
    waT = nc.dram_tensor("waT", [C, 3 * FPC], bf16, kind="ExternalInput")
    wpsh = nc.dram_tensor("wpsh", [128, C], bf16, kind="ExternalInput")
    cs32 = nc.dram_tensor("cs32", [32, T], f32, kind="ExternalInput")
    sn32 = nc.dram_tensor("sn32", [32, T], f32, kind="ExternalInput")
    qw = nc.dram_tensor("qw", [128, 1], f32, kind="ExternalInput")
    kw = nc.dram_tensor("kw", [128, 1], f32, kind="ExternalInput")
    bones = nc.dram_tensor("bones", [128, 2], f32, kind="ExternalInput")
    sel2 = nc.dram_tensor("sel2", [2, 128], f32, kind="ExternalInput")
    wedge = nc.dram_tensor("wedge", [128, 128], f32, kind="ExternalInput")
    ident = nc.dram_tensor("ident", [128, 128], f32, kind="ExternalInput")
    vones = nc.dram_tensor("vones", [128, 32], f32, kind="ExternalInput")
    out = nc.dram_tensor("out", [BT // N_CORES, C], f16, kind="ExternalOutput")

    with tile.TileContext(nc) as tc:
        with (
            tc.tile_pool(name="const", bufs=1) as const,
            tc.tile_pool(name="resid", bufs=1) as resid,
            tc.tile_pool(name="xtp", bufs=6) as xtp,
            tc.tile_pool(name="work", bufs=3) as work,
            tc.tile_pool(name="pwork", bufs=4) as pwork,
            tc.tile_pool(name="mm", bufs=2, space="PSUM") as mmp,
            tc.tile_pool(name="yp", bufs=2, space="PSUM") as ypp,
            tc.tile_pool(name="sp", bufs=1, space="PSUM") as spp,
            tc.tile_pool(name="bcp", bufs=1, space="PSUM") as bcp,
            tc.tile_pool(name="dram", bufs=1, space="DRAM") as dramp,
        ):
            # ---- on-device re-replication of host-sharded inputs ----
            agx_in = dramp.tile([C, 512], bf16, tag="agx_in")
            agxA = dramp.tile([N_CORES, 512, 512], bf16, tag="agxA",
                              addr_space="Shared")
            agxB = dramp.tile([N_CORES, 512, 512], bf16, tag="agxB",
                              addr_space="Shared")
            agw_in = dramp.tile([128, C], bf16, tag="agw_in")
            agw = dramp.tile([N_CORES, 128, C], bf16, tag="agw",
                             addr_space="Shared")

            nc.sync.dma_start(agx_in[:, :], xsh.ap())
            nc.sync.dma_start(agw_in[:, :], wpsh.ap())
            if no_cc:
                for r in range(N_CORES):
                    nc.sync.dma_start(agxA[r], agx_in[0:512, :])
                    nc.sync.dma_start(agxB[r], agx_in[512:1024, :])
                    nc.sync.dma_start(agw[r], agw_in[:, :])
            else:
                nc.gpsimd.collective_compute(
                    "AllGather", BYPASS, replica_groups=RG,
                    ins=[agx_in[0:512, :].opt()], outs=[agxA[:, :, :].opt()])
                nc.gpsimd.collective_compute(
                    "AllGather", BYPASS, replica_groups=RG,
                    ins=[agx_in[512:1024, :].opt()], outs=[agxB[:, :, :].opt()])
                nc.gpsimd.collective_compute(
                    "AllGather", BYPASS, replica_groups=RG,
                    ins=[agw_in[:, :].opt()], outs=[agw[:, :, :].opt()])

            # ---- constants to SBUF ----
            wa_sb = const.tile([128, C // 128, 3 * FPC], bf16, tag="wa")
            nc.sync.dma_start(wa_sb[:], waT.ap().rearrange("(o p) f -> p o f", p=128))
            qw_sb = const.tile([128, 1], f32, tag="qw")
            nc.sync.dma_start(qw_sb[:], qw[:, :])
            kw_sb = const.tile([128, 1], f32, tag="kw")
            nc.sync.dma_start(kw_sb[:], kw[:, :])
            bo_sb = const.tile([128, 2], f32r, tag="bo")
            nc.sync.dma_start(bo_sb[:], r32(bones[:, :]))
            s2_sb = const.tile([2, 128], f32r, tag="s2")
            nc.sync.dma_start(s2_sb[:], r32(sel2[:, :]))
            id_sb = const.tile([128, 128], f32, tag="id")
            nc.sync.dma_start(id_sb[:], ident[:, :])
            eps_sb = const.tile([128, 1], f32, tag="eps")
            nc.vector.memset(eps_sb[:], EPS)
            cs_sb = const.tile([128, T], f32, tag="cs")
            sn_sb = const.tile([128, T], f32, tag="sn")
            wg_sb = const.tile([128, 128], f32, tag="wg")

            def emit_late_consts():
                nc.sync.dma_start(vA[:, :, HD], r32(vones[:, :]))
                nc.sync.dma_start(vA[:, :, 2 * HD + 1], r32(vones[:, :]))
                for b0 in (0, 32, 64, 96):
                    nc.sync.dma_start(cs_sb[b0:b0 + 32, :], cs32[:, :])
                    nc.sync.dma_start(sn_sb[b0:b0 + 32, :], sn32[:, :])
                # sign pattern [-sn, sn, -sn, sn] built in place
                nc.scalar.mul(sn_sb[0:32, :], sn_sb[0:32, :], -1.0)
                nc.scalar.mul(sn_sb[64:96, :], sn_sb[64:96, :], -1.0)
                nc.sync.dma_start(wg_sb[:], wedge[:, :])

            # ---- residents ----
            qT = resid.tile([128, BT], f32r, tag="qT")   # roped+normed q^T
            kT = resid.tile([128, BT], f32r, tag="kT")
            # attention out^T, both heads packed [128, BT]; written via
            # SBUF->SBUF DMA (cross-partition moves are DMA-only)
            yHp = resid.tile([128, BT], bf16, tag="yHp")
            # V in token-major + ones cols: per head h: cols [65h:65h+64]=V_h,
            # col 65h+64 = 1.0
            vA = resid.tile([128, BT // 128, 2 * (HD + 1)], f32r, tag="vA")

            # ================= QKV + RMSNorm + RoPE =================
            xts = {}

            def emit_xt(n):
                xtA = xtp.tile([128, 4, 512], bf16, tag="xt", name=f"xtA{n}")
                nc.sync.dma_start(
                    xtA[:], agxA[n].rearrange("(o p) t -> p o t", p=128))
                xtB = xtp.tile([128, 4, 512], bf16, tag="xt", name=f"xtB{n}")
                nc.sync.dma_start(
                    xtB[:], agxB[n].rearrange("(o p) t -> p o t", p=128))
                xts[n] = (xtA, xtB)

            def emit_qkv(n):
                tok = slice(512 * n, 512 * n + 512)
                ct = slice(512 * (n % 4), 512 * (n % 4) + 512)
                if n not in xts:
                    emit_xt(n)
                xtA, xtB = xts.pop(n)

                bigQK = mmp.tile([128, 1024], f32, tag="big", name=f"qk{n}")
                bigV = mmp.tile([128, 1024], f32, tag="big", name=f"v{n}")
                for m, dst, wcol in ((0, qT, qw_sb), (1, kT, kw_sb), (2, None, None)):
                    ps = bigV[:, 0:512] if m == 2 else bigQK[:, 512 * m:512 * m + 512]
                    for kt in range(C // 128):
                        nc.tensor.matmul(
                            ps,
                            wa_sb[:, kt, 128 * m:128 * m + 128],
                            xtA[:, kt, :] if kt < 4 else xtB[:, kt - 4, :],
                            start=(kt == 0), stop=(kt == C // 128 - 1),
                        )
                    if m == 2:
                        # V: token-major via PE transpose of 128x128 blocks
                        vs = work.tile([128, 512], f32, tag="vs", name=f"vs{n}")
                        nc.scalar.copy(vs[:], ps)
                        for j in range(4):
                            pt = spp.tile([128, 128], f32, tag="sm", name=f"vt{n}_{j}")
                            nc.tensor.transpose(pt[:], vs[:, 128 * j:128 * j + 128],
                                                id_sb[:])
                            kt_g = 4 * n + j
                            nc.vector.tensor_copy(
                                vA[:, kt_g].rearrange("p (h d) -> p h d", h=2)[:, :, 0:HD],
                                pt[:, :].rearrange("p (h d) -> p h d", h=2))
                        continue

                    # stats from raw (pre-weight) psum
                    sq = work.tile([128, 512], f32, tag="scr", name=f"sq{n}_{m}")
                    nc.scalar.activation(r32(sq[:]), ps, AF.Square)
                    ss = spp.tile([2, 512], f32, tag="sm", name=f"ss{n}_{m}")
                    nc.tensor.matmul(ss[:], r32(bo_sb[:]), r32(sq[:]),
                                     start=True, stop=True)
                    inv = work.tile([2, 512], f32, tag="rms", name=f"rms{n}_{m}")
                    nc.scalar.activation(r32(inv[:]), ss[:], AF.Sqrt,
                                         bias=eps_sb[0:2, :], scale=1.0 / HD)
                    with nc.allow_low_precision(reason="f32r is fp32-width"):
                        nc.vector.reciprocal(r32(inv[:]), inv[:])

                    # apply norm weight on the way out of PSUM
                    nc.vector.tensor_scalar_mul(dst[:, tok], ps, wcol[:])

                    # rope: r = q*CS + swap(q)*SN  (swap halves within head)
                    sw = work.tile([128, 512], f32r, tag="sw", name=f"sw{n}_{m}")
                    for h in range(HPC):
                        b0 = 64 * h
                        nc.sync.dma_start(sw[b0:b0 + 32, :], dst[b0 + 32:b0 + 64, tok])
                        nc.sync.dma_start(sw[b0 + 32:b0 + 64, :], dst[b0:b0 + 32, tok])
                    nc.vector.tensor_tensor(sw[:], sw[:], sn_sb[:, ct], MUL)
                    nc.vector.tensor_tensor(dst[:, tok], dst[:, tok], cs_sb[:, ct], MUL)
                    nc.vector.tensor_tensor(dst[:, tok], dst[:, tok], sw[:], ADD)

                    # apply 1/rms: broadcast [2,512] -> [128,512] via K=2 matmul
                    bc = bcp.tile([128, 512], f32, tag="bc", name=f"bc{n}_{m}")
                    nc.tensor.matmul(bc[:], r32(s2_sb[:]), r32(inv[:]),
                                     start=True, stop=True)
                    nc.vector.tensor_tensor(r32(dst[:, tok]), dst[:, tok], bc[:], MUL)

            # ================= causal attention =================
            a_in = dramp.tile([N_CORES, 128, 512], bf16, tag="a_in")

            def emit_attn(b, i):
                if True:
                    qcol = slice(2048 * b + 512 * i, 2048 * b + 512 * i + 512)
                    nkt = 4 * i + 4
                    yps = [ypp.tile([HD + 1, 512], f32, tag="y",
                                    name=f"y{b}_{i}_{h}") for h in range(HPC)]
                    for kt in range(nkt):
                        qs = 128 * (kt - 4 * i) if kt >= 4 * i else 0
                        kc = 2048 * b + 128 * kt
                        kt_g = 16 * b + kt
                        sps = mmp.tile([128, 1024], f32, tag="big",
                                       name=f"s{b}_{i}_{kt}")
                        pT = pwork.tile([128, 1024], f32, tag="pT",
                                        name=f"p{b}_{i}_{kt}")
                        for h in range(HPC):
                            hb = 64 * h
                            nc.tensor.matmul(
                                sps[:, 512 * h + qs:512 * h + 512],
                                r32(kT[hb:hb + 64, kc:kc + 128]),
                                r32(qT[hb:hb + 64, qcol][:, qs:]),
                                start=True, stop=True,
                                tile_position=(hb, 0),
                            )
                        sps3 = sps[:, :].rearrange("p (h q) -> p h q", h=2)[:, :, qs:]
                        pT3 = pT[:, :].rearrange("p (h q) -> p h q", h=2)[:, :, qs:]
                        nc.scalar.activation(r32(pT3), sps3, AF.Exp,
                                             scale=1.0 / 8.0)
                        for h in range(HPC):
                            if kt >= 4 * i:
                                nc.vector.tensor_tensor(
                                    r32(pT[:, 512 * h + qs:512 * h + qs + 128]),
                                    pT[:, 512 * h + qs:512 * h + qs + 128],
                                    wg_sb[:], MUL)
                            nc.tensor.matmul(
                                yps[h][:, qs:],
                                r32(vA[:, kt_g, (HD + 1) * h:(HD + 1) * h + HD + 1]),
                                r32(pT[:, 512 * h + qs:512 * h + 512]),
                                start=(kt == 0), stop=(kt == nkt - 1),
                            )
                    # normalize by the ones-column denominator
                    for h in range(HPC):
                        di = work.tile([1, 512], f32, tag="rms",
                                       name=f"di{b}_{i}_{h}")
                        with nc.allow_low_precision(reason="f32r is fp32-width"):
                            nc.vector.reciprocal(r32(di[:]), yps[h][HD:HD + 1, :])
                        dp = spp.tile([64, 512], f32, tag="sm",
                                      name=f"dp{b}_{i}_{h}")
                        nc.tensor.matmul(dp[:], r32(s2_sb[0:1, 0:64]), r32(di[:]),
                                         start=True, stop=True)
                        dpS = work.tile([64, 512], f32, tag="dpS",
                                        name=f"dpS{b}_{i}_{h}")
                        nc.scalar.copy(dpS[:], dp[:])
                        ybf = work.tile([HD, 512], bf16, tag="ybf",
                                        name=f"ybf{b}_{i}_{h}")
                        nc.vector.tensor_tensor(ybf[:, :],
                                                yps[h][:HD, :], dpS[:, :],
                                                MUL)
                        nc.sync.dma_start(yHp[64 * h:64 * h + HD, qcol],
                                          ybf[:, :])
                    nc.sync.dma_start(a_in[4 * b + i], yHp[:, qcol])

            emit_xt(0)
            emit_late_consts()
            emit_qkv(0)
            for n in range(1, TN // 2):
                emit_qkv(n)
                emit_attn(0, n - 1)
            wp_sb = resid.tile([128, N_CORES, 1024], bf16, tag="wp_sb")
            nc.sync.dma_start(wp_sb[:], agw[:, :, :].rearrange("o p f -> p o f"))
            emit_qkv(TN // 2)
            emit_attn(0, 3)
            for n in range(TN // 2 + 1, TN):
                emit_qkv(n)
                emit_attn(1, n - TN // 2 - 1)
            emit_attn(1, 3)

            # ================= AllToAll reshard =================
            a_out = dramp.tile([N_CORES, 128, 512], bf16, tag="a_out")
            if no_cc:
                nc.sync.dma_start(a_out[:, :, :], a_in[:, :, :])
            else:
                nc.gpsimd.collective_compute(
                    "AllToAll", BYPASS, replica_groups=RG,
                    ins=[a_in[:, :, :].opt()], outs=[a_out[:, :, :].opt()])

            # ================= c_proj on own 512-token slice =================
            ybr = resid.tile([128, N_CORES, 512], bf16, tag="ybr")
            for r in range(N_CORES):
                nc.sync.dma_start(ybr[:, r, :], a_out[r])
            for cc in range(2):
                ccol = slice(512 * cc, 512 * cc + 512)
                bigP = [mmp.tile([128, 1024], f32, tag="big",
                                 name=f"po{cc}_{t}") for t in range(2)]
                pouts = [bigP[t // 2][:, 512 * (t % 2):512 * (t % 2) + 512]
                         for t in range(4)]
                for r in range(N_CORES):
                    for t in range(4):
                        nc.tensor.matmul(
                            pouts[t],
                            ybr[:, r, 128 * t:128 * t + 128],
                            wp_sb[:, r, ccol],
                            start=(r == 0), stop=(r == N_CORES - 1),
                        )
                for t in range(4):
                    ob = work.tile([128, 512], f16, tag="obf", name=f"ob{cc}_{t}")
                    nc.scalar.copy(ob[:], pouts[t])
                    nc.sync.dma_start(out[128 * t:128 * t + 128, ccol], ob[:])

    nc.compile()
    return nc


def make_in_maps(x, freqs_cos, freqs_sin, w_attn, w_proj, q_norm_w, k_norm_w):
    x = np.asarray(x, np.float32)
    freqs_cos = np.asarray(freqs_cos, np.float32)
    freqs_sin = np.asarray(freqs_sin, np.float32)
    w_attn = np.asarray(w_attn, np.float32)
    w_proj = np.asarray(w_proj, np.float32)
    q_norm_w = np.asarray(q_norm_w, np.float32)
    k_norm_w = np.asarray(k_norm_w, np.float32)

    perm = np.concatenate([np.arange(0, HD, 2), np.arange(1, HD, 2)])
    import ml_dtypes
    xTf = np.ascontiguousarray(x.reshape(BT, C).T.astype(ml_dtypes.bfloat16))
    wpT = np.ascontiguousarray(w_proj.T.astype(ml_dtypes.bfloat16))

    cs32 = np.ascontiguousarray(freqs_cos.T)  # [32, T]
    sn32 = np.ascontiguousarray(freqs_sin.T)

    qwc = np.tile(q_norm_w[perm], HPC)[:, None].astype(np.float32)
    kwc = np.tile(k_norm_w[perm], HPC)[:, None].astype(np.float32)

    bones = np.zeros((128, 2), np.float32)
    bones[:64, 0] = 1.0
    bones[64:, 1] = 1.0
    sel2 = np.zeros((2, 128), np.float32)
    sel2[0, :64] = 1.0
    sel2[1, 64:] = 1.0
    wedge = (np.arange(128)[:, None] <= np.arange(128)[None, :]).astype(np.float32)
    vones = np.ones((128, 32), np.float32)
    ident = np.eye(128, dtype=np.float32)

    in_maps = []
    for c in range(N_CORES):
        rows = []
        for sec in range(3):  # q, k, v sections of w_attn
            for h in (HPC * c, HPC * c + 1):
                base = C * sec + HD * h
                if sec < 2:
                    rows.append(base + perm)
                else:
                    rows.append(base + np.arange(HD))
        sel_rows = np.concatenate(rows)
        waT = np.ascontiguousarray(w_attn[sel_rows].T.astype(ml_dtypes.bfloat16))
        xsh = np.ascontiguousarray(xTf[:, 512 * c:512 * c + 512])
        wpsh = np.ascontiguousarray(wpT[128 * c:128 * c + 128, :])
        in_maps.append({
            "xsh": xsh, "waT": waT, "wpsh": wpsh, "cs32": cs32, "sn32": sn32,
            "qw": qwc, "kw": kwc, "bones": bones, "sel2": sel2,
            "wedge": wedge, "ident": ident, "vones": vones,
        })
    return in_maps


_NC_CACHE = {}


def get_nc():
    if "nc" not in _NC_CACHE:
        _NC_CACHE["nc"] = build_nc()
    return _NC_CACHE["nc"]


def kernel(x, freqs_cos, freqs_sin, w_attn, w_proj, q_norm_w, k_norm_w):
    nc = get_nc()
    in_maps = make_in_maps(x, freqs_cos, freqs_sin, w_attn, w_proj,
                           q_norm_w, k_norm_w)
    res = run_bass_kernel_spmd(nc, in_maps, core_ids=list(range(N_CORES)))
    out = np.concatenate([res.results[c]["out"] for c in range(N_CORES)], axis=0)
    return out.reshape(B, T, C).astype(np.float32)


# revision 3
# speedup vs baseline: 1.9850x; 1.9850x over previous
"""Causal self-attention (B=2,T=2048,C=1024,H=16,hd=64) with QK-RMSNorm + RoPE.

8-core Trainium2 Bass kernel. Sharding: tensor-parallel over heads (2 heads per
core) for QKV + attention, then an AllToAll reshards the attention output
token-wise so each core computes the exact c_proj output for its 512-token
slice (no partial sums, no all-reduce).

Host->device traffic is the measured bottleneck (per-dispatch staging at
~17 GB/s), so every replicated input is sharded on the host and re-replicated
on device over the much faster inter-core links:
  - x ships token-sharded (1MB/core) and is AllGathered on device (two
    feature-half AllGathers so QKV matmuls can start after the first half).
  - w_proj ships row-sharded (0.25MB/core) and is AllGathered on device.
  - cos/sin ship as [32, T] once (not 4x-duplicated rows); the 128-row SBUF
    tiling and sign pattern are built on device.
  - the output is written in fp16 (2e-2 rel tolerance leaves plenty of room).

Layout strategy: everything feature-major ("transposed") on device.
  - per-core waT = w_attn[sel_rows].T so QKV matmuls produce qT/kT/vT
    [feat, tok] with no on-device activation transposes.
  - q,k feature order is permuted to [evens, odds] per head (host-side weight
    row permutation) which turns interleaved RoPE into half-block ops; S = q.k
    is invariant to the shared permutation.
  - S^T tiles [keys,queries] come from lhsT=kT, rhs=qT; softmax denominator is
    computed by a ones-column appended to V (scores are bounded: |s| <= 8
    after RMS-norm, so exp needs no max subtraction).
  - gpsimd runs ONLY collectives (AllGather x2, AllGather wp, AllToAll);
    element-wise work that used to ride gpsimd now rides the vector engine so
    collectives are never queued behind it.
"""

import numpy as np

import concourse.bass as bass
import concourse.mybir as mybir
import concourse.tile as tile
from concourse import bacc
from concourse.bass_utils import run_bass_kernel_spmd

B, T, C = 2, 2048, 1024
H, HD = 16, 64
N_CORES = 8
HPC = H // N_CORES  # heads per core = 2
BT = B * T  # 4096 flattened tokens
FPC = HPC * HD  # feats per core = 128
EPS = 1e-6
TN = BT // 512  # 8 token tiles of 512
QB = T // 512  # 4 query blocks per sequence

f32 = mybir.dt.float32
f32r = mybir.dt.float32r
f16 = mybir.dt.float16
bf16 = mybir.dt.bfloat16
MUL = mybir.AluOpType.mult
ADD = mybir.AluOpType.add
AF = mybir.ActivationFunctionType
BYPASS = mybir.AluOpType.bypass

RG = [list(range(N_CORES))]


def r32(ap):
    return ap.bitcast(f32r)


def build_nc(single_core=False, no_cc=False):
    no_cc = no_cc or single_core
    nc = bacc.Bacc("TRN2", target_bir_lowering=False, debug=False,
                   num_devices=1 if single_core else N_CORES)

    xsh = nc.dram_tensor("xsh", [C, 512], bf16, kind="ExternalInput")
    waT = nc.dram_tensor("waT", [C, 3 * FPC], bf16, kind="ExternalInput")
    wpsh = nc.dram_tensor("wpsh", [128, C], bf16, kind="ExternalInput")
    cs32 = nc.dram_tensor("cs32", [32, T], f32, kind="ExternalInput")
    sn32 = nc.dram_tensor("sn32", [32, T], f32, kind="ExternalInput")
    qw = nc.dram_tensor("qw", [128, 1], f32, kind="ExternalInput")
    kw = nc.dram_tensor("kw", [128, 1], f32, kind="ExternalInput")
    bones = nc.dram_tensor("bones", [128, 2], f32, kind="ExternalInput")
    sel2 = nc.dram_tensor("sel2", [2, 128], f32, kind="ExternalInput")
    wedge = nc.dram_tensor("wedge", [128, 128], f32, kind="ExternalInput")
    ident = nc.dram_tensor("ident", [128, 128], f32, kind="ExternalInput")
    vones = nc.dram_tensor("vones", [128, 32], f32, kind="ExternalInput")
    out = nc.dram_tensor("out", [BT // N_CORES, C], f16, kind="ExternalOutput")

    with tile.TileContext(nc) as tc:
        with (
            tc.tile_pool(name="const", bufs=1) as const,
            tc.tile_pool(name="resid", bufs=1) as resid,
            tc.tile_pool(name="xtp", bufs=6) as xtp,
            tc.tile_pool(name="work", bufs=3) as work,
            tc.tile_pool(name="pwork", bufs=4) as pwork,
            tc.tile_pool(name="mm", bufs=2, space="PSUM") as mmp,
            tc.tile_pool(name="yp", bufs=2, space="PSUM") as ypp,
            tc.tile_pool(name="sp", bufs=1, space="PSUM") as spp,
            tc.tile_pool(name="bcp", bufs=1, space="PSUM") as bcp,
            tc.tile_pool(name="dram", bufs=1, space="DRAM") as dramp,
        ):
            # ---- on-device re-replication of host-sharded inputs ----
            aspace = "Local" if no_cc else "Shared"
            agx_in = dramp.tile([C, 512], bf16, tag="agx_in")
            agxA = dramp.tile([N_CORES, 512, 512], bf16, tag="agxA",
                              addr_space=aspace)
            agxB = dramp.tile([N_CORES, 512, 512], bf16, tag="agxB",
                              addr_space=aspace)
            agw_in = dramp.tile([128, C], bf16, tag="agw_in")
            agw = dramp.tile([N_CORES, 128, C], bf16, tag="agw",
                             addr_space=aspace)

            nc.sync.dma_start(agx_in[:, :], xsh.ap())
            nc.sync.dma_start(agw_in[:, :], wpsh.ap())
            if no_cc:
                for r in range(N_CORES):
                    nc.sync.dma_start(agxA[r], agx_in[0:512, :])
                    nc.sync.dma_start(agxB[r], agx_in[512:1024, :])
                    nc.sync.dma_start(agw[r], agw_in[:, :])
            else:
                nc.gpsimd.collective_compute(
                    "AllGather", BYPASS, replica_groups=RG,
                    ins=[agx_in[0:512, :].opt()], outs=[agxA[:, :, :].opt()])
                nc.gpsimd.collective_compute(
                    "AllGather", BYPASS, replica_groups=RG,
                    ins=[agx_in[512:1024, :].opt()], outs=[agxB[:, :, :].opt()])
                nc.gpsimd.collective_compute(
                    "AllGather", BYPASS, replica_groups=RG,
                    ins=[agw_in[:, :].opt()], outs=[agw[:, :, :].opt()])

            # ---- constants to SBUF ----
            wa_sb = const.tile([128, C // 128, 3 * FPC], bf16, tag="wa")
            nc.sync.dma_start(wa_sb[:], waT.ap().rearrange("(o p) f -> p o f", p=128))
            qw_sb = const.tile([128, 1], f32, tag="qw")
            nc.sync.dma_start(qw_sb[:], qw[:, :])
            kw_sb = const.tile([128, 1], f32, tag="kw")
            nc.sync.dma_start(kw_sb[:], kw[:, :])
            bo_sb = const.tile([128, 2], f32r, tag="bo")
            nc.sync.dma_start(bo_sb[:], r32(bones[:, :]))
            s2_sb = const.tile([2, 128], f32r, tag="s2")
            nc.sync.dma_start(s2_sb[:], r32(sel2[:, :]))
            id_sb = const.tile([128, 128], f32, tag="id")
            nc.sync.dma_start(id_sb[:], ident[:, :])
            eps_sb = const.tile([128, 1], f32, tag="eps")
            nc.vector.memset(eps_sb[:], EPS)
            cs_sb = const.tile([128, T], f32, tag="cs")
            sn_sb = const.tile([128, T], f32, tag="sn")
            wg_sb = const.tile([128, 128], f32, tag="wg")

            def emit_late_consts():
                nc.sync.dma_start(vA[:, :, HD], r32(vones[:, :]))
                nc.sync.dma_start(vA[:, :, 2 * HD + 1], r32(vones[:, :]))
                for b0 in (0, 32, 64, 96):
                    nc.sync.dma_start(cs_sb[b0:b0 + 32, :], cs32[:, :])
                    nc.sync.dma_start(sn_sb[b0:b0 + 32, :], sn32[:, :])
                # sign pattern [-sn, sn, -sn, sn] built in place
                nc.scalar.mul(sn_sb[0:32, :], sn_sb[0:32, :], -1.0)
                nc.scalar.mul(sn_sb[64:96, :], sn_sb[64:96, :], -1.0)
                nc.sync.dma_start(wg_sb[:], wedge[:, :])

            # ---- residents ----
            qT = resid.tile([128, BT], f32r, tag="qT")   # roped+normed q^T
            kT = resid.tile([128, BT], f32r, tag="kT")
            # attention out^T, both heads packed [128, BT]; written via
            # SBUF->SBUF DMA (cross-partition moves are DMA-only)
            yHp = resid.tile([128, BT], bf16, tag="yHp")
            # V in token-major + ones cols: per head h: cols [65h:65h+64]=V_h,
            # col 65h+64 = 1.0
            vA = resid.tile([128, BT // 128, 2 * (HD + 1)], f32r, tag="vA")

            # ================= QKV + RMSNorm + RoPE =================
            xts = {}

            def emit_xt(n):
                xtA = xtp.tile([128, 4, 512], bf16, tag="xt", name=f"xtA{n}")
                nc.sync.dma_start(
                    xtA[:], agxA[n].rearrange("(o p) t -> p o t", p=128))
                xtB = xtp.tile([128, 4, 512], bf16, tag="xt", name=f"xtB{n}")
                nc.sync.dma_start(
                    xtB[:], agxB[n].rearrange("(o p) t -> p o t", p=128))
                xts[n] = (xtA, xtB)

            def emit_qkv(n):
                tok = slice(512 * n, 512 * n + 512)
                ct = slice(512 * (n % 4), 512 * (n % 4) + 512)
                if n not in xts:
                    emit_xt(n)
                xtA, xtB = xts.pop(n)

                bigQK = mmp.tile([128, 1024], f32, tag="big", name=f"qk{n}")
                bigV = mmp.tile([128, 1024], f32, tag="big", name=f"v{n}")
                for m, dst, wcol in ((0, qT, qw_sb), (1, kT, kw_sb), (2, None, None)):
                    ps = bigV[:, 0:512] if m == 2 else bigQK[:, 512 * m:512 * m + 512]
                    for kt in range(C // 128):
                        nc.tensor.matmul(
                            ps,
                            wa_sb[:, kt, 128 * m:128 * m + 128],
                            xtA[:, kt, :] if kt < 4 else xtB[:, kt - 4, :],
                            start=(kt == 0), stop=(kt == C // 128 - 1),
                        )
                    if m == 2:
                        # V: token-major via PE transpose of 128x128 blocks
                        vs = work.tile([128, 512], f32, tag="vs", name=f"vs{n}")
                        nc.scalar.copy(vs[:], ps)
                        for j in range(4):
                            pt = spp.tile([128, 128], f32, tag="sm", name=f"vt{n}_{j}")
                            nc.tensor.transpose(pt[:], vs[:, 128 * j:128 * j + 128],
                                                id_sb[:])
                            kt_g = 4 * n + j
                            nc.vector.tensor_copy(
                                vA[:, kt_g].rearrange("p (h d) -> p h d", h=2)[:, :, 0:HD],
                                pt[:, :].rearrange("p (h d) -> p h d", h=2))
                        continue

                    # stats from raw (pre-weight) psum
                    sq = work.tile([128, 512], f32, tag="scr", name=f"sq{n}_{m}")
                    nc.scalar.activation(r32(sq[:]), ps, AF.Square)
                    ss = spp.tile([2, 512], f32, tag="sm", name=f"ss{n}_{m}")
                    nc.tensor.matmul(ss[:], r32(bo_sb[:]), r32(sq[:]),
                                     start=True, stop=True)
                    inv = work.tile([2, 512], f32, tag="rms", name=f"rms{n}_{m}")
                    nc.scalar.activation(r32(inv[:]), ss[:], AF.Sqrt,
                                         bias=eps_sb[0:2, :], scale=1.0 / HD)
                    with nc.allow_low_precision(reason="f32r is fp32-width"):
                        nc.vector.reciprocal(r32(inv[:]), inv[:])

                    # apply norm weight on the way out of PSUM
                    nc.vector.tensor_scalar_mul(dst[:, tok], ps, wcol[:])

                    # rope: r = q*CS + swap(q)*SN  (swap halves within head)
                    sw = work.tile([128, 512], f32r, tag="sw", name=f"sw{n}_{m}")
                    for h in range(HPC):
                        b0 = 64 * h
                        nc.sync.dma_start(sw[b0:b0 + 32, :], dst[b0 + 32:b0 + 64, tok])
                        nc.sync.dma_start(sw[b0 + 32:b0 + 64, :], dst[b0:b0 + 32, tok])
                    nc.vector.tensor_tensor(sw[:], sw[:], sn_sb[:, ct], MUL)
                    nc.vector.tensor_tensor(dst[:, tok], dst[:, tok], cs_sb[:, ct], MUL)
                    nc.vector.tensor_tensor(dst[:, tok], dst[:, tok], sw[:], ADD)

                    # apply 1/rms: broadcast [2,512] -> [128,512] via K=2 matmul
                    bc = bcp.tile([128, 512], f32, tag="bc", name=f"bc{n}_{m}")
                    nc.tensor.matmul(bc[:], r32(s2_sb[:]), r32(inv[:]),
                                     start=True, stop=True)
                    nc.vector.tensor_tensor(r32(dst[:, tok]), dst[:, tok], bc[:], MUL)

            # ================= causal attention =================
            a_in = dramp.tile([N_CORES, 128, 512], bf16, tag="a_in")

            def emit_attn(b, i):
                if True:
                    qcol = slice(2048 * b + 512 * i, 2048 * b + 512 * i + 512)
                    nkt = 4 * i + 4
                    yps = [ypp.tile([HD + 1, 512], f32, tag="y",
                                    name=f"y{b}_{i}_{h}") for h in range(HPC)]
                    for kt in range(nkt):
                        qs = 128 * (kt - 4 * i) if kt >= 4 * i else 0
                        kc = 2048 * b + 128 * kt
                        kt_g = 16 * b + kt
                        sps = mmp.tile([128, 1024], f32, tag="big",
                                       name=f"s{b}_{i}_{kt}")
                        pT = pwork.tile([128, 1024], f32, tag="pT",
                                        name=f"p{b}_{i}_{kt}")
                        for h in range(HPC):
                            hb = 64 * h
                            nc.tensor.matmul(
                                sps[:, 512 * h + qs:512 * h + 512],
                                r32(kT[hb:hb + 64, kc:kc + 128]),
                                r32(qT[hb:hb + 64, qcol][:, qs:]),
                                start=True, stop=True,
                                tile_position=(hb, 0),
                            )
                        sps3 = sps[:, :].rearrange("p (h q) -> p h q", h=2)[:, :, qs:]
                        pT3 = pT[:, :].rearrange("p (h q) -> p h q", h=2)[:, :, qs:]
                        nc.scalar.activation(r32(pT3), sps3, AF.Exp,
                                             scale=1.0 / 8.0)
                        for h in range(HPC):
                            if kt >= 4 * i:
                                nc.vector.tensor_tensor(
                                    r32(pT[:, 512 * h + qs:512 * h + qs + 128]),
                                    pT[:, 512 * h + qs:512 * h + qs + 128],
                                    wg_sb[:], MUL)
                            nc.tensor.matmul(
                                yps[h][:, qs:],
                                r32(vA[:, kt_g, (HD + 1) * h:(HD + 1) * h + HD + 1]),
                                r32(pT[:, 512 * h + qs:512 * h + 512]),
                                start=(kt == 0), stop=(kt == nkt - 1),
                            )
                    # normalize by the ones-column denominator
                    for h in range(HPC):
                        di = work.tile([1, 512], f32, tag="rms",
                                       name=f"di{b}_{i}_{h}")
                        with nc.allow_low_precision(reason="f32r is fp32-width"):
                            nc.vector.reciprocal(r32(di[:]), yps[h][HD:HD + 1, :])
                        dp = spp.tile([64, 512], f32, tag="sm",
                                      name=f"dp{b}_{i}_{h}")
                        nc.tensor.matmul(dp[:], r32(s2_sb[0:1, 0:64]), r32(di[:]),
                                         start=True, stop=True)
                        dpS = work.tile([64, 512], f32, tag="dpS",
                                        name=f"dpS{b}_{i}_{h}")
                        nc.scalar.copy(dpS[:], dp[:])
                        ybf = work.tile([HD, 512], bf16, tag="ybf",
                                        name=f"ybf{b}_{i}_{h}")
                        nc.vector.tensor_tensor(ybf[:, :],
                                                yps[h][:HD, :], dpS[:, :],
                                                MUL)
                        nc.sync.dma_start(yHp[64 * h:64 * h + HD, qcol],
                                          ybf[:, :])
                    nc.sync.dma_start(a_in[4 * b + i], yHp[:, qcol])

            emit_xt(0)
            emit_late_consts()
            emit_qkv(0)
            for n in range(1, TN // 2):
                emit_qkv(n)
                emit_attn(0, n - 1)
            wp_sb = resid.tile([128, N_CORES, 1024], bf16, tag="wp_sb")
            nc.sync.dma_start(wp_sb[:], agw[:, :, :].rearrange("o p f -> p o f"))
            emit_qkv(TN // 2)
            emit_attn(0, 3)
            for n in range(TN // 2 + 1, TN):
                emit_qkv(n)
                emit_attn(1, n - TN // 2 - 1)
            emit_attn(1, 3)

            # ================= AllToAll reshard =================
            a_out = dramp.tile([N_CORES, 128, 512], bf16, tag="a_out")
            if no_cc:
                nc.sync.dma_start(a_out[:, :, :], a_in[:, :, :])
            else:
                nc.gpsimd.collective_compute(
                    "AllToAll", BYPASS, replica_groups=RG,
                    ins=[a_in[:, :, :].opt()], outs=[a_out[:, :, :].opt()])

            # ================= c_proj on own 512-token slice =================
            ybr = resid.tile([128, N_CORES, 512], bf16, tag="ybr")
            for r in range(N_CORES):
                nc.sync.dma_start(ybr[:, r, :], a_out[r])
            for cc in range(2):
                ccol = slice(512 * cc, 512 * cc + 512)
                bigP = [mmp.tile([128, 1024], f32, tag="big",
                                 name=f"po{cc}_{t}") for t in range(2)]
                pouts = [bigP[t // 2][:, 512 * (t % 2):512 * (t % 2) + 512]
                         for t in range(4)]
                for r in range(N_CORES):
                    for t in range(4):
                        nc.tensor.matmul(
                            pouts[t],
                            ybr[:, r, 128 * t:128 * t + 128],
                            wp_sb[:, r, ccol],
                            start=(r == 0), stop=(r == N_CORES - 1),
                        )
                for t in range(4):
                    ob = work.tile([128, 512], f16, tag="obf", name=f"ob{cc}_{t}")
                    nc.scalar.copy(ob[:], pouts[t])
                    nc.sync.dma_start(out[128 * t:128 * t + 128, ccol], ob[:])

    nc.compile()
    return nc


def make_in_maps(x, freqs_cos, freqs_sin, w_attn, w_proj, q_norm_w, k_norm_w):
    x = np.asarray(x, np.float32)
    freqs_cos = np.asarray(freqs_cos, np.float32)
    freqs_sin = np.asarray(freqs_sin, np.float32)
    w_attn = np.asarray(w_attn, np.float32)
    w_proj = np.asarray(w_proj, np.float32)
    q_norm_w = np.asarray(q_norm_w, np.float32)
    k_norm_w = np.asarray(k_norm_w, np.float32)

    perm = np.concatenate([np.arange(0, HD, 2), np.arange(1, HD, 2)])
    import ml_dtypes
    xTf = np.ascontiguousarray(x.reshape(BT, C).T.astype(ml_dtypes.bfloat16))
    wpT = np.ascontiguousarray(w_proj.T.astype(ml_dtypes.bfloat16))

    cs32 = np.ascontiguousarray(freqs_cos.T)  # [32, T]
    sn32 = np.ascontiguousarray(freqs_sin.T)

    qwc = np.tile(q_norm_w[perm], HPC)[:, None].astype(np.float32)
    kwc = np.tile(k_norm_w[perm], HPC)[:, None].astype(np.float32)

    bones = np.zeros((128, 2), np.float32)
    bones[:64, 0] = 1.0
    bones[64:, 1] = 1.0
    sel2 = np.zeros((2, 128), np.float32)
    sel2[0, :64] = 1.0
    sel2[1, 64:] = 1.0
    wedge = (np.arange(128)[:, None] <= np.arange(128)[None, :]).astype(np.float32)
    vones = np.ones((128, 32), np.float32)
    ident = np.eye(128, dtype=np.float32)

    in_maps = []
    for c in range(N_CORES):
        rows = []
        for sec in range(3):  # q, k, v sections of w_attn
            for h in (HPC * c, HPC * c + 1):
                base = C * sec + HD * h
                if sec < 2:
                    rows.append(base + perm)
                else:
                    rows.append(base + np.arange(HD))
        sel_rows = np.concatenate(rows)
        waT = np.ascontiguousarray(w_attn[sel_rows].T.astype(ml_dtypes.bfloat16))
        xsh = np.ascontiguousarray(xTf[:, 512 * c:512 * c + 512])
        wpsh = np.ascontiguousarray(wpT[128 * c:128 * c + 128, :])
        in_maps.append({
            "xsh": xsh, "waT": waT, "wpsh": wpsh, "cs32": cs32, "sn32": sn32,
            "qw": qwc, "kw": kwc, "bones": bones, "sel2": sel2,
            "wedge": wedge, "ident": ident, "vones": vones,
        })
    return in_maps


_NC_CACHE = {}


def get_nc():
    if "nc" not in _NC_CACHE:
        _NC_CACHE["nc"] = build_nc()
    return _NC_CACHE["nc"]


def kernel(x, freqs_cos, freqs_sin, w_attn, w_proj, q_norm_w, k_norm_w):
    nc = get_nc()
    in_maps = make_in_maps(x, freqs_cos, freqs_sin, w_attn, w_proj,
                           q_norm_w, k_norm_w)
    res = run_bass_kernel_spmd(nc, in_maps, core_ids=list(range(N_CORES)))
    out = np.concatenate([res.results[c]["out"] for c in range(N_CORES)], axis=0)
    return out.reshape(B, T, C).astype(np.float32)


# revision 4
# speedup vs baseline: 9.6196x; 4.8461x over previous
"""Causal self-attention (B=2,T=2048,C=1024,H=16,hd=64) with QK-RMSNorm + RoPE.

8-core Trainium2 Bass kernel. Sharding: tensor-parallel over heads (2 heads per
core) for QKV + attention, then an AllToAll reshards the attention output
token-wise so each core computes the exact c_proj output for its 512-token
slice (no partial sums, no all-reduce).

Host->device traffic is the measured bottleneck (per-dispatch staging at
~17 GB/s), so every replicated input is sharded on the host and re-replicated
on device over the much faster inter-core links:
  - x ships token-sharded (1MB/core) and is AllGathered on device (two
    feature-half AllGathers so QKV matmuls can start after the first half).
  - w_proj ships row-sharded (0.25MB/core) and is AllGathered on device.
  - cos/sin ship as [32, T] once (not 4x-duplicated rows); the 128-row SBUF
    tiling and sign pattern are built on device.
  - the output is written in fp16 (2e-2 rel tolerance leaves plenty of room).

Layout strategy: everything feature-major ("transposed") on device.
  - per-core waT = w_attn[sel_rows].T so QKV matmuls produce qT/kT/vT
    [feat, tok] with no on-device activation transposes.
  - q,k feature order is permuted to [evens, odds] per head (host-side weight
    row permutation) which turns interleaved RoPE into half-block ops; S = q.k
    is invariant to the shared permutation.
  - S^T tiles [keys,queries] come from lhsT=kT, rhs=qT; softmax denominator is
    computed by a ones-column appended to V (scores are bounded: |s| <= 8
    after RMS-norm, so exp needs no max subtraction).
  - gpsimd runs ONLY collectives (AllGather x2, AllGather wp, AllToAll);
    element-wise work that used to ride gpsimd now rides the vector engine so
    collectives are never queued behind it.
"""

import numpy as np

import concourse.bass as bass
import concourse.mybir as mybir
import concourse.tile as tile
from concourse import bacc
from concourse.bass_utils import run_bass_kernel_spmd

B, T, C = 2, 2048, 1024
H, HD = 16, 64
N_CORES = 8
HPC = H // N_CORES  # heads per core = 2
BT = B * T  # 4096 flattened tokens
FPC = HPC * HD  # feats per core = 128
EPS = 1e-6
TN = BT // 512  # 8 token tiles of 512
QB = T // 512  # 4 query blocks per sequence

# packed single-input blob layout (bf16 columns, [128, NB])
OX = 0            # x shard, p-major [128, 8*512]
OWA = 4096        # w_attn shard.T, p-major [128, 8*384]
OWP = 7168        # w_proj.T rows shard [128, 1024]
OCS = 8192        # rows 0:32 cs [32,2048] f32; rows 32:64 sn
OQW = 12288       # [128,1] f32
OKW = 12290
OBO = 12292       # bones [128,2] f32
OS2 = 12296       # sel2 [2,128] f32 (rows 0:2)
OWG = 12552       # wedge [128,128] f32
OID = 12808       # ident [128,128] f32
OVO = 13064       # vones [128,32] f32
NB = 13312

f32 = mybir.dt.float32
f32r = mybir.dt.float32r
f16 = mybir.dt.float16
bf16 = mybir.dt.bfloat16
MUL = mybir.AluOpType.mult
ADD = mybir.AluOpType.add
AF = mybir.ActivationFunctionType
BYPASS = mybir.AluOpType.bypass

RG = [list(range(N_CORES))]


def r32(ap):
    return ap.bitcast(f32r)


def build_nc(single_core=False, no_cc=False):
    no_cc = no_cc or single_core
    nc = bacc.Bacc("TRN2", target_bir_lowering=False, debug=False,
                   num_devices=1 if single_core else N_CORES)

    blob = nc.dram_tensor("blob", [128, NB], bf16, kind="ExternalInput")
    out = nc.dram_tensor("out", [BT // N_CORES, C], f16, kind="ExternalOutput")

    bap = blob.ap()
    x_ap = bap[:, OX:OX + 4096]
    wa_ap = bap[:, OWA:OWA + 3072].rearrange("p (o f) -> p o f", o=8)
    wp_ap = bap[:, OWP:OWP + 1024]
    cs_ap = bap[0:32, OCS:OCS + 4096].bitcast(f32)
    sn_ap = bap[32:64, OCS:OCS + 4096].bitcast(f32)
    qw_ap = bap[:, OQW:OQW + 2].bitcast(f32)
    kw_ap = bap[:, OKW:OKW + 2].bitcast(f32)
    bo_ap = bap[:, OBO:OBO + 4].bitcast(f32r)
    s2_ap = bap[0:2, OS2:OS2 + 256].bitcast(f32r)
    wg_ap = bap[:, OWG:OWG + 256].bitcast(f32)
    id_ap = bap[:, OID:OID + 256].bitcast(f32)
    vo_ap = bap[:, OVO:OVO + 64].bitcast(f32r)

    with tile.TileContext(nc) as tc:
        with (
            tc.tile_pool(name="const", bufs=1) as const,
            tc.tile_pool(name="resid", bufs=1) as resid,
            tc.tile_pool(name="xtp", bufs=6) as xtp,
            tc.tile_pool(name="work", bufs=3) as work,
            tc.tile_pool(name="pwork", bufs=4) as pwork,
            tc.tile_pool(name="mm", bufs=2, space="PSUM") as mmp,
            tc.tile_pool(name="yp", bufs=2, space="PSUM") as ypp,
            tc.tile_pool(name="sp", bufs=1, space="PSUM") as spp,
            tc.tile_pool(name="bcp", bufs=1, space="PSUM") as bcp,
            tc.tile_pool(name="dram", bufs=1, space="DRAM") as dramp,
        ):
            # ---- on-device re-replication of host-sharded inputs ----
            aspace = "Local" if no_cc else "Shared"
            agxA_in = dramp.tile([128, 2048], bf16, tag="agxA_in")
            agxB_in = dramp.tile([128, 2048], bf16, tag="agxB_in")
            agxA = dramp.tile([N_CORES, 128, 2048], bf16, tag="agxA",
                              addr_space=aspace)
            agxB = dramp.tile([N_CORES, 128, 2048], bf16, tag="agxB",
                              addr_space=aspace)
            agw_in = dramp.tile([128, C], bf16, tag="agw_in")
            agw = dramp.tile([N_CORES, 128, C], bf16, tag="agw",
                             addr_space=aspace)

            nc.sync.dma_start(agxA_in[:, :], x_ap[:, 0:2048])
            nc.sync.dma_start(agxB_in[:, :], x_ap[:, 2048:4096])
            nc.sync.dma_start(agw_in[:, :], wp_ap)
            if no_cc:
                for r in range(N_CORES):
                    nc.sync.dma_start(agxA[r], agxA_in[:, :])
                    nc.sync.dma_start(agxB[r], agxB_in[:, :])
                    nc.sync.dma_start(agw[r], agw_in[:, :])
            else:
                nc.gpsimd.collective_compute(
                    "AllGather", BYPASS, replica_groups=RG,
                    ins=[agxA_in[:, :].opt()], outs=[agxA[:, :, :].opt()])
                nc.gpsimd.collective_compute(
                    "AllGather", BYPASS, replica_groups=RG,
                    ins=[agxB_in[:, :].opt()], outs=[agxB[:, :, :].opt()])
                nc.gpsimd.collective_compute(
                    "AllGather", BYPASS, replica_groups=RG,
                    ins=[agw_in[:, :].opt()], outs=[agw[:, :, :].opt()])

            # ---- constants to SBUF ----
            wa_sb = const.tile([128, C // 128, 3 * FPC], bf16, tag="wa")
            nc.sync.dma_start(wa_sb[:], wa_ap)
            qw_sb = const.tile([128, 1], f32, tag="qw")
            nc.sync.dma_start(qw_sb[:], qw_ap)
            kw_sb = const.tile([128, 1], f32, tag="kw")
            nc.sync.dma_start(kw_sb[:], kw_ap)
            bo_sb = const.tile([128, 2], f32r, tag="bo")
            nc.sync.dma_start(bo_sb[:], bo_ap)
            s2_sb = const.tile([2, 128], f32r, tag="s2")
            nc.sync.dma_start(s2_sb[:], s2_ap)
            id_sb = const.tile([128, 128], f32, tag="id")
            nc.sync.dma_start(id_sb[:], id_ap)
            eps_sb = const.tile([128, 1], f32, tag="eps")
            nc.vector.memset(eps_sb[:], EPS)
            cs_sb = const.tile([128, T], f32, tag="cs")
            sn_sb = const.tile([128, T], f32, tag="sn")
            wg_sb = const.tile([128, 128], f32, tag="wg")

            def emit_late_consts():
                nc.sync.dma_start(vA[:, :, HD], vo_ap)
                nc.sync.dma_start(vA[:, :, 2 * HD + 1], vo_ap)
                for b0 in (0, 32, 64, 96):
                    nc.sync.dma_start(cs_sb[b0:b0 + 32, :], cs_ap)
                    nc.sync.dma_start(sn_sb[b0:b0 + 32, :], sn_ap)
                # sign pattern [-sn, sn, -sn, sn] built in place
                nc.scalar.mul(sn_sb[0:32, :], sn_sb[0:32, :], -1.0)
                nc.scalar.mul(sn_sb[64:96, :], sn_sb[64:96, :], -1.0)
                nc.sync.dma_start(wg_sb[:], wg_ap)

            # ---- residents ----
            qT = resid.tile([128, BT], f32r, tag="qT")   # roped+normed q^T
            kT = resid.tile([128, BT], f32r, tag="kT")
            # attention out^T, both heads packed [128, BT]; written via
            # SBUF->SBUF DMA (cross-partition moves are DMA-only)
            yHp = resid.tile([128, BT], bf16, tag="yHp")
            # V in token-major + ones cols: per head h: cols [65h:65h+64]=V_h,
            # col 65h+64 = 1.0
            vA = resid.tile([128, BT // 128, 2 * (HD + 1)], f32r, tag="vA")

            # ================= QKV + RMSNorm + RoPE =================
            xts = {}

            def emit_xt(n):
                xtA = xtp.tile([128, 4, 512], bf16, tag="xt", name=f"xtA{n}")
                nc.sync.dma_start(
                    xtA[:], agxA[n].rearrange("p (o t) -> p o t", o=4))
                xtB = xtp.tile([128, 4, 512], bf16, tag="xt", name=f"xtB{n}")
                nc.sync.dma_start(
                    xtB[:], agxB[n].rearrange("p (o t) -> p o t", o=4))
                xts[n] = (xtA, xtB)

            def emit_qkv(n):
                tok = slice(512 * n, 512 * n + 512)
                ct = slice(512 * (n % 4), 512 * (n % 4) + 512)
                if n not in xts:
                    emit_xt(n)
                xtA, xtB = xts.pop(n)

                bigQK = mmp.tile([128, 1024], f32, tag="big", name=f"qk{n}")
                bigV = mmp.tile([128, 1024], f32, tag="big", name=f"v{n}")
                for m, dst, wcol in ((0, qT, qw_sb), (1, kT, kw_sb), (2, None, None)):
                    ps = bigV[:, 0:512] if m == 2 else bigQK[:, 512 * m:512 * m + 512]
                    for kt in range(C // 128):
                        nc.tensor.matmul(
                            ps,
                            wa_sb[:, kt, 128 * m:128 * m + 128],
                            xtA[:, kt, :] if kt < 4 else xtB[:, kt - 4, :],
                            start=(kt == 0), stop=(kt == C // 128 - 1),
                        )
                    if m == 2:
                        # V: token-major via PE transpose of 128x128 blocks
                        vs = work.tile([128, 512], f32, tag="vs", name=f"vs{n}")
                        nc.scalar.copy(vs[:], ps)
                        for j in range(4):
                            pt = spp.tile([128, 128], f32, tag="sm", name=f"vt{n}_{j}")
                            nc.tensor.transpose(pt[:], vs[:, 128 * j:128 * j + 128],
                                                id_sb[:])
                            kt_g = 4 * n + j
                            nc.vector.tensor_copy(
                                vA[:, kt_g].rearrange("p (h d) -> p h d", h=2)[:, :, 0:HD],
                                pt[:, :].rearrange("p (h d) -> p h d", h=2))
                        continue

                    # stats from raw (pre-weight) psum
                    sq = work.tile([128, 512], f32, tag="scr", name=f"sq{n}_{m}")
                    nc.scalar.activation(r32(sq[:]), ps, AF.Square)
                    ss = spp.tile([2, 512], f32, tag="sm", name=f"ss{n}_{m}")
                    nc.tensor.matmul(ss[:], r32(bo_sb[:]), r32(sq[:]),
                                     start=True, stop=True)
                    inv = work.tile([2, 512], f32, tag="rms", name=f"rms{n}_{m}")
                    nc.scalar.activation(r32(inv[:]), ss[:], AF.Sqrt,
                                         bias=eps_sb[0:2, :], scale=1.0 / HD)
                    with nc.allow_low_precision(reason="f32r is fp32-width"):
                        nc.vector.reciprocal(r32(inv[:]), inv[:])

                    # apply norm weight on the way out of PSUM
                    nc.vector.tensor_scalar_mul(dst[:, tok], ps, wcol[:])

                    # rope: r = q*CS + swap(q)*SN  (swap halves within head)
                    sw = work.tile([128, 512], f32r, tag="sw", name=f"sw{n}_{m}")
                    for h in range(HPC):
                        b0 = 64 * h
                        nc.sync.dma_start(sw[b0:b0 + 32, :], dst[b0 + 32:b0 + 64, tok])
                        nc.sync.dma_start(sw[b0 + 32:b0 + 64, :], dst[b0:b0 + 32, tok])
                    nc.vector.tensor_tensor(sw[:], sw[:], sn_sb[:, ct], MUL)
                    nc.vector.tensor_tensor(dst[:, tok], dst[:, tok], cs_sb[:, ct], MUL)
                    nc.vector.tensor_tensor(dst[:, tok], dst[:, tok], sw[:], ADD)

                    # apply 1/rms: broadcast [2,512] -> [128,512] via K=2 matmul
                    bc = bcp.tile([128, 512], f32, tag="bc", name=f"bc{n}_{m}")
                    nc.tensor.matmul(bc[:], r32(s2_sb[:]), r32(inv[:]),
                                     start=True, stop=True)
                    nc.vector.tensor_tensor(r32(dst[:, tok]), dst[:, tok], bc[:], MUL)

            # ================= causal attention =================
            a_in = dramp.tile([N_CORES, 128, 512], bf16, tag="a_in")

            def emit_attn(b, i):
                if True:
                    qcol = slice(2048 * b + 512 * i, 2048 * b + 512 * i + 512)
                    nkt = 4 * i + 4
                    yps = [ypp.tile([HD + 1, 512], f32, tag="y",
                                    name=f"y{b}_{i}_{h}") for h in range(HPC)]
                    for kt in range(nkt):
                        qs = 128 * (kt - 4 * i) if kt >= 4 * i else 0
                        kc = 2048 * b + 128 * kt
                        kt_g = 16 * b + kt
                        sps = mmp.tile([128, 1024], f32, tag="big",
                                       name=f"s{b}_{i}_{kt}")
                        pT = pwork.tile([128, 1024], f32, tag="pT",
                                        name=f"p{b}_{i}_{kt}")
                        for h in range(HPC):
                            hb = 64 * h
                            nc.tensor.matmul(
                                sps[:, 512 * h + qs:512 * h + 512],
                                r32(kT[hb:hb + 64, kc:kc + 128]),
                                r32(qT[hb:hb + 64, qcol][:, qs:]),
                                start=True, stop=True,
                                tile_position=(hb, 0),
                            )
                        sps3 = sps[:, :].rearrange("p (h q) -> p h q", h=2)[:, :, qs:]
                        pT3 = pT[:, :].rearrange("p (h q) -> p h q", h=2)[:, :, qs:]
                        nc.scalar.activation(r32(pT3), sps3, AF.Exp,
                                             scale=1.0 / 8.0)
                        for h in range(HPC):
                            if kt >= 4 * i:
                                nc.vector.tensor_tensor(
                                    r32(pT[:, 512 * h + qs:512 * h + qs + 128]),
                                    pT[:, 512 * h + qs:512 * h + qs + 128],
                                    wg_sb[:], MUL)
                            nc.tensor.matmul(
                                yps[h][:, qs:],
                                r32(vA[:, kt_g, (HD + 1) * h:(HD + 1) * h + HD + 1]),
                                r32(pT[:, 512 * h + qs:512 * h + 512]),
                                start=(kt == 0), stop=(kt == nkt - 1),
                            )
                    # normalize by the ones-column denominator
                    for h in range(HPC):
                        di = work.tile([1, 512], f32, tag="rms",
                                       name=f"di{b}_{i}_{h}")
                        with nc.allow_low_precision(reason="f32r is fp32-width"):
                            nc.vector.reciprocal(r32(di[:]), yps[h][HD:HD + 1, :])
                        dp = spp.tile([64, 512], f32, tag="sm",
                                      name=f"dp{b}_{i}_{h}")
                        nc.tensor.matmul(dp[:], r32(s2_sb[0:1, 0:64]), r32(di[:]),
                                         start=True, stop=True)
                        dpS = work.tile([64, 512], f32, tag="dpS",
                                        name=f"dpS{b}_{i}_{h}")
                        nc.scalar.copy(dpS[:], dp[:])
                        ybf = work.tile([HD, 512], bf16, tag="ybf",
                                        name=f"ybf{b}_{i}_{h}")
                        nc.vector.tensor_tensor(ybf[:, :],
                                                yps[h][:HD, :], dpS[:, :],
                                                MUL)
                        nc.sync.dma_start(yHp[64 * h:64 * h + HD, qcol],
                                          ybf[:, :])
                    nc.sync.dma_start(a_in[4 * b + i], yHp[:, qcol])

            emit_xt(0)
            emit_late_consts()
            emit_qkv(0)
            for n in range(1, TN // 2):
                emit_qkv(n)
                emit_attn(0, n - 1)
            wp_sb = resid.tile([128, N_CORES, 1024], bf16, tag="wp_sb")
            nc.sync.dma_start(wp_sb[:], agw[:, :, :].rearrange("o p f -> p o f"))
            emit_qkv(TN // 2)
            emit_attn(0, 3)
            for n in range(TN // 2 + 1, TN):
                emit_qkv(n)
                emit_attn(1, n - TN // 2 - 1)
            emit_attn(1, 3)

            # ================= AllToAll reshard =================
            a_out = dramp.tile([N_CORES, 128, 512], bf16, tag="a_out")
            if no_cc:
                nc.sync.dma_start(a_out[:, :, :], a_in[:, :, :])
            else:
                nc.gpsimd.collective_compute(
                    "AllToAll", BYPASS, replica_groups=RG,
                    ins=[a_in[:, :, :].opt()], outs=[a_out[:, :, :].opt()])

            # ================= c_proj on own 512-token slice =================
            ybr = resid.tile([128, N_CORES, 512], bf16, tag="ybr")
            for r in range(N_CORES):
                nc.sync.dma_start(ybr[:, r, :], a_out[r])
            for cc in range(2):
                ccol = slice(512 * cc, 512 * cc + 512)
                bigP = [mmp.tile([128, 1024], f32, tag="big",
                                 name=f"po{cc}_{t}") for t in range(2)]
                pouts = [bigP[t // 2][:, 512 * (t % 2):512 * (t % 2) + 512]
                         for t in range(4)]
                for r in range(N_CORES):
                    for t in range(4):
                        nc.tensor.matmul(
                            pouts[t],
                            ybr[:, r, 128 * t:128 * t + 128],
                            wp_sb[:, r, ccol],
                            start=(r == 0), stop=(r == N_CORES - 1),
                        )
                for t in range(4):
                    ob = work.tile([128, 512], f16, tag="obf", name=f"ob{cc}_{t}")
                    nc.scalar.copy(ob[:], pouts[t])
                    nc.sync.dma_start(out[128 * t:128 * t + 128, ccol], ob[:])

    nc.compile()
    return nc


def make_in_maps(x, freqs_cos, freqs_sin, w_attn, w_proj, q_norm_w, k_norm_w):
    x = np.asarray(x, np.float32)
    freqs_cos = np.asarray(freqs_cos, np.float32)
    freqs_sin = np.asarray(freqs_sin, np.float32)
    w_attn = np.asarray(w_attn, np.float32)
    w_proj = np.asarray(w_proj, np.float32)
    q_norm_w = np.asarray(q_norm_w, np.float32)
    k_norm_w = np.asarray(k_norm_w, np.float32)

    perm = np.concatenate([np.arange(0, HD, 2), np.arange(1, HD, 2)])
    import ml_dtypes
    bfloat16 = ml_dtypes.bfloat16
    xTf = np.ascontiguousarray(x.reshape(BT, C).T.astype(bfloat16))
    wpT = np.ascontiguousarray(w_proj.T.astype(bfloat16))

    cs32 = np.ascontiguousarray(freqs_cos.T).astype(np.float32)  # [32, T]
    sn32 = np.ascontiguousarray(freqs_sin.T).astype(np.float32)

    qwc = np.tile(q_norm_w[perm], HPC)[:, None].astype(np.float32)
    kwc = np.tile(k_norm_w[perm], HPC)[:, None].astype(np.float32)

    bones = np.zeros((128, 2), np.float32)
    bones[:64, 0] = 1.0
    bones[64:, 1] = 1.0
    sel2 = np.zeros((2, 128), np.float32)
    sel2[0, :64] = 1.0
    sel2[1, 64:] = 1.0
    wedge = (np.arange(128)[:, None] <= np.arange(128)[None, :]).astype(np.float32)
    vones = np.ones((128, 32), np.float32)
    ident = np.eye(128, dtype=np.float32)

    in_maps = []
    for c in range(N_CORES):
        rows = []
        for sec in range(3):  # q, k, v sections of w_attn
            for h in (HPC * c, HPC * c + 1):
                base = C * sec + HD * h
                if sec < 2:
                    rows.append(base + perm)
                else:
                    rows.append(base + np.arange(HD))
        sel_rows = np.concatenate(rows)
        waT = np.ascontiguousarray(w_attn[sel_rows].T.astype(bfloat16))
        xsh = xTf[:, 512 * c:512 * c + 512]  # [1024, 512]
        wpsh = wpT[128 * c:128 * c + 128, :]

        bb = np.zeros((128, NB), bfloat16)

        def putf32(arr, r0, c0):
            v = np.ascontiguousarray(arr.astype(np.float32)).view(bfloat16)
            bb[r0:r0 + v.shape[0], c0:c0 + v.shape[1]] = v

        bb[:, OX:OX + 4096] = (
            xsh.reshape(8, 128, 512).transpose(1, 0, 2).reshape(128, 4096))
        bb[:, OWA:OWA + 3072] = (
            waT.reshape(8, 128, 3 * FPC).transpose(1, 0, 2).reshape(128, 3072))
        bb[:, OWP:OWP + 1024] = wpsh
        putf32(cs32, 0, OCS)
        putf32(sn32, 32, OCS)
        putf32(qwc, 0, OQW)
        putf32(kwc, 0, OKW)
        putf32(bones, 0, OBO)
        putf32(sel2, 0, OS2)
        putf32(wedge, 0, OWG)
        putf32(ident, 0, OID)
        putf32(vones, 0, OVO)
        in_maps.append({"blob": bb})
    return in_maps


_NC_CACHE = {}


def get_nc():
    if "nc" not in _NC_CACHE:
        _NC_CACHE["nc"] = build_nc()
    return _NC_CACHE["nc"]


def kernel(x, freqs_cos, freqs_sin, w_attn, w_proj, q_norm_w, k_norm_w):
    nc = get_nc()
    in_maps = make_in_maps(x, freqs_cos, freqs_sin, w_attn, w_proj,
                           q_norm_w, k_norm_w)
    res = run_bass_kernel_spmd(nc, in_maps, core_ids=list(range(N_CORES)))
    out = np.concatenate([res.results[c]["out"] for c in range(N_CORES)], axis=0)
    return out.reshape(B, T, C).astype(np.float32)
